# revision 20
# baseline (speedup 1.0000x reference)
"""Bahdanau-attention kernel for Trainium2, data-parallel over batch on 8 NeuronCores.

Per-core shard: 8 batches (1568 flat encoder rows). Design:
  - enc is read from HBM exactly once: SWDGE cast-DMA (f32->bf16) into
    resident SBUF "nat" tiles (one per chunk). PE-transposes produce encT
    (E on partitions) interleaved with the main matmuls so the PE stays warm.
  - W1/W2 are cast to bf16 DRAM staging slices, then loaded transposed via
    the xbar DMA-transpose (few, large instructions; no PE time).
  - Main loop per flat-row chunk [384,384,384,416]: bf16 matmuls accumulate
    enc_projT in f32 PSUM -> ScalarE tanh with per-partition bias
    Hb = h_projT + W1_b + W2_b -> V-dot matmul accumulates scores (f32).
  - Per completed batch: f32 softmax on its scores row; attn row DMA'd out;
    w row cast/scattered into the block-diagonal W8 [8 x 1568].
  - End: W8 column blocks are PE-transposed to wdiag [128 x 8] chunks; the
    context for all 8 batches accumulates in one [8, 512] PSUM per e-chunk
    from the resident nat tiles (enc never re-read).
"""

import sys

sys.path.insert(0, "/opt/trn_rl_repo")
sys.path.insert(0, "/opt/pypackages")

import numpy as np
from contextlib import ExitStack

import concourse.bass as bass
import concourse.bacc as bacc
import concourse.mybir as mybir
import concourse.tile as tile
from concourse.masks import make_identity

F32 = mybir.dt.float32
BF16 = mybir.dt.bfloat16
AF = mybir.ActivationFunctionType
ALU = mybir.AluOpType
AX = mybir.AxisListType

B, P, E, A = 64, 196, 2048, 1024
NCORES = 8
BL = B // NCORES          # 8 local batches per core
BP = BL * P               # 1568 flat rows per core
KE = E // 128             # 16 contraction chunks
KA = A // 128             # 8 attn-dim tiles
CHUNKS = [(0, 384), (384, 384), (768, 384), (1152, 416)]
CW = 416                  # max chunk width
NT = (BP + 127) // 128    # 13 flat 128-row tiles


def _segments(S, W):
    segs = []
    b = S // P
    pos = S
    while pos < S + W:
        end = min((b + 1) * P, S + W)
        segs.append((b, pos - S, end - pos))
        pos = end
        b += 1
    return segs


def _done_batches():
    done = []
    prev = 0
    for S, W in CHUNKS:
        nb = (S + W) // P
        done.append(list(range(prev, nb)))
        prev = nb
    return done


DONE = _done_batches()


def build_nc():
    nc = bacc.Bacc("TRN2", target_bir_lowering=False, debug=False,
                   enable_asserts=False)

    enc = nc.dram_tensor("enc_hiddens", [BL, P, E], F32, kind="ExternalInput")
    dec = nc.dram_tensor("dec_prev_hidden", [BL, E], F32, kind="ExternalInput")
    w1 = nc.dram_tensor("W1_w", [A, E], F32, kind="ExternalInput")
    w1b = nc.dram_tensor("W1_b", [A], F32, kind="ExternalInput")
    w2 = nc.dram_tensor("W2_w", [A, E], F32, kind="ExternalInput")
    w2b = nc.dram_tensor("W2_b", [A], F32, kind="ExternalInput")
    vw = nc.dram_tensor("V_w", [1, A], F32, kind="ExternalInput")
    out_ctx = nc.dram_tensor("out_ctx", [BL, E], F32, kind="ExternalOutput")
    out_attn = nc.dram_tensor("out_attn", [BL, P], F32, kind="ExternalOutput")

    enc_flat = enc.ap().rearrange("b p e -> (b p) e")

    with tile.TileContext(nc) as tc:
        with ExitStack() as ctx:
            const = ctx.enter_context(tc.tile_pool(name="const", bufs=1))
            identb = const.tile([128, 128], BF16)
            make_identity(nc, identb[:])

            # Persistent SBUF tensors
            w2t = const.tile([128, KE * A], BF16)       # W2^T: [e, k-major a]
            hb = const.tile([128, KA * BL], F32)        # Hb[q, a*BL+b]
            vt = const.tile([128, KA], BF16)            # V^T columns per a-tile
            scores = const.tile([1, BP], F32)
            w8 = const.tile([BL, BP], BF16)             # block-diag softmax w
            nc.vector.memset(w8[:], 0.0)
            # resident bf16 enc rows, one tile per chunk: [128, nj*E]
            nats = []
            for ci, (S, W) in enumerate(CHUNKS):
                nj = (W + 127) // 128
                nats.append(const.tile([128, nj * E], BF16, name=f"nat{ci}"))

            tp_ps = ctx.enter_context(
                tc.tile_pool(name="tp_ps", bufs=2, space="PSUM"))

            dramp = ctx.enter_context(
                tc.tile_pool(name="dram", bufs=1, space="DRAM"))
            w2stage = [dramp.tile([A, 512], BF16, name=f"w2stage{q}")
                       for q in range(4)]
            w1stage = [dramp.tile([A, 512], BF16, name=f"w1stage{q}")
                       for q in range(4)]

            # ---- cast-DMA issue order (one SWDGE queue, roughly serial) ----
            def cast_chunk(ci):
                S, W = CHUNKS[ci]
                nfull = W // 128
                nc.gpsimd.dma_start(
                    nats[ci][:].rearrange("p (j e) -> p j e", e=E)[:, :nfull, :],
                    enc_flat[S:S + nfull * 128, :]
                    .rearrange("(j p) e -> p j e", p=128))
                rem = W - nfull * 128
                if rem:
                    nc.gpsimd.dma_start(
                        nats[ci][:rem, nfull * E:nfull * E + E],
                        enc_flat[S + nfull * 128:S + W, :])

            cast_chunk(0)
            for q in range(4):
                nc.gpsimd.dma_start(w1stage[q][:],
                                    w1.ap()[:, q * 512:(q + 1) * 512])
            cast_chunk(1)
            for q in range(4):
                nc.gpsimd.dma_start(w2stage[q][:],
                                    w2.ap()[:, q * 512:(q + 1) * 512])
            cast_chunk(2)
            cast_chunk(3)

            # ---------------- setup ----------------
            with ExitStack() as sctx:
                sp1 = sctx.enter_context(tc.tile_pool(name="setup1", bufs=1))
                hps_pool = sctx.enter_context(
                    tc.tile_pool(name="hps", bufs=2, space="PSUM"))

                # W1T / W2T via DRAM->SBUF xbar DMA transpose (W1 first: Hb
                # is needed at the first tanh).
                w2t3 = w2t[:].rearrange("p (k a) -> p k a", k=KE)
                w1t = sp1.tile([128, KE * A], BF16)
                w1t3 = w1t[:].rearrange("p (k a) -> p k a", k=KE)
                for q in range(4):
                    nc.scalar.dma_start_transpose(
                        w1t3[:, 4 * q: 4 * (q + 1), :], w1stage[q][:])
                for q in range(4):
                    nc.scalar.dma_start_transpose(
                        w2t3[:, 4 * q: 4 * (q + 1), :], w2stage[q][:])

                # dec -> decT (bf16)
                dec_sb = sp1.tile([BL, E], BF16)
                nc.gpsimd.dma_start(dec_sb[:], dec.ap())
                dect = sp1.tile([128, KE * BL], BF16)
                for k in range(KE):
                    ps = tp_ps.tile([128, 128], BF16, tag="tp")
                    nc.tensor.transpose(
                        ps[:, :BL], dec_sb[:, k * 128:(k + 1) * 128],
                        identb[:BL, :BL])
                    nc.any.tensor_copy(dect[:, k * BL:(k + 1) * BL], ps[:, :BL])

                # h_proj = dec @ W1.T  -> [BL, A] f32
                h_sb = sp1.tile([BL, A], F32)
                for half in range(2):
                    hps = hps_pool.tile([BL, 512], F32, tag="hps")
                    for k in range(KE):
                        nc.tensor.matmul(
                            hps[:],
                            dect[:, k * BL:(k + 1) * BL],
                            w1t[:, k * A + half * 512: k * A + half * 512 + 512],
                            start=(k == 0), stop=(k == KE - 1))
                    nc.any.tensor_copy(h_sb[:, half * 512:(half + 1) * 512],
                                       hps[:])

                # bias columns: W1_b + W2_b, laid out [q, a]  (f32)
                w1bc = sp1.tile([128, KA], F32)
                w2bc = sp1.tile([128, KA], F32)
                with nc.allow_non_contiguous_dma(reason="tiny transposed loads"):
                    nc.sync.dma_start(
                        w1bc[:], w1b.ap().rearrange("(a k) -> k a", k=128))
                    nc.sync.dma_start(
                        w2bc[:], w2b.ap().rearrange("(a k) -> k a", k=128))
                    nc.gpsimd.dma_start(
                        vt[:], vw.ap().rearrange("o (a k) -> k (o a)", k=128))
                nc.vector.tensor_add(w1bc[:], w1bc[:], w2bc[:])

                # hT + bias -> Hb  (bf16 transpose of h_sb via PE)
                h_bf = sp1.tile([BL, A], BF16)
                nc.vector.tensor_copy(h_bf[:], h_sb[:])
                for a in range(KA):
                    ps = tp_ps.tile([128, 128], BF16, tag="tp")
                    nc.tensor.transpose(
                        ps[:, :BL], h_bf[:, a * 128:(a + 1) * 128],
                        identb[:BL, :BL])
                    nc.vector.tensor_scalar(
                        out=hb[:, a * BL:(a + 1) * BL], in0=ps[:, :BL],
                        scalar1=w1bc[:, a:a + 1], scalar2=None, op0=ALU.add)

            # ---------------- main loop ----------------
            enctp = ctx.enter_context(tc.tile_pool(name="enct", bufs=2))
            tpool = ctx.enter_context(tc.tile_pool(name="tpool", bufs=3))
            smp = ctx.enter_context(tc.tile_pool(name="smp", bufs=2))
            bigps = ctx.enter_context(
                tc.tile_pool(name="bigps", bufs=3, space="PSUM"))
            scps = ctx.enter_context(
                tc.tile_pool(name="scps", bufs=1, space="PSUM"))

            for ci, (S, W) in enumerate(CHUNKS):
                nat = nats[ci]
                enct = enctp.tile([128, KE * CW], BF16, tag="enct")
                nj = (W + 127) // 128
                for j in range(nj):
                    rows = min(128, W - j * 128)
                    for k in range(KE):
                        ps = tp_ps.tile([128, 128], BF16, tag="tp")
                        nc.tensor.transpose(
                            ps[:, :rows],
                            nat[:rows, j * E + k * 128: j * E + (k + 1) * 128],
                            identb[:rows, :rows])
                        nc.vector.tensor_copy(
                            enct[:, k * CW + j * 128: k * CW + j * 128 + rows],
                            ps[:, :rows])

                segs = _segments(S, W)
                sc = scps.tile([1, CW], F32, tag="sc")
                for a in range(KA):
                    ps = bigps.tile([128, CW], F32, tag="big")
                    for k in range(KE):
                        nc.tensor.matmul(
                            ps[:, :W],
                            w2t[:, k * A + a * 128: k * A + (a + 1) * 128],
                            enct[:, k * CW: k * CW + W],
                            start=(k == 0), stop=(k == KE - 1))
                    t_sb = tpool.tile([128, CW], BF16, tag="t")
                    for (b, off, ln) in segs:
                        nc.scalar.activation(
                            t_sb[:, off:off + ln], ps[:, off:off + ln], AF.Tanh,
                            bias=hb[:, a * BL + b: a * BL + b + 1])
                    nc.tensor.matmul(
                        sc[:, :W], vt[:, a:a + 1], t_sb[:, :W],
                        start=(a == 0), stop=(a == KA - 1))
                nc.any.tensor_copy(scores[:, S:S + W], sc[:, :W])

                for b in DONE[ci]:
                    sseg = scores[:, b * P:(b + 1) * P]
                    negmax = smp.tile([1, 1], F32, tag="nm")
                    nc.vector.reduce_max(negmax[:], sseg, axis=AX.X, negate=True)
                    ex = smp.tile([1, P], F32, tag="ex")
                    nc.scalar.activation(ex[:], sseg, AF.Exp, bias=negmax[:])
                    ssum = smp.tile([1, 1], F32, tag="sm")
                    nc.vector.reduce_sum(ssum[:], ex[:], axis=AX.X)
                    rcp = smp.tile([1, 1], F32, tag="rc")
                    nc.vector.reciprocal(rcp[:], ssum[:])
                    wsb = smp.tile([1, P], F32, tag="w")
                    nc.vector.tensor_scalar(
                        out=wsb[:], in0=ex[:], scalar1=rcp[:], scalar2=None,
                        op0=ALU.mult)
                    nc.sync.dma_start(out_attn.ap()[b:b + 1, :], wsb[:])
                    # scatter w row (cast to bf16) onto the block diagonal
                    nc.gpsimd.dma_start(
                        w8[b:b + 1, b * P:(b + 1) * P], wsb[:])

            # ---------------- context (end phase) ----------------
            ctxps = ctx.enter_context(
                tc.tile_pool(name="ctxps", bufs=2, space="PSUM"))
            ctxp = ctx.enter_context(tc.tile_pool(name="ctxsb", bufs=1))
            wdiag = const.tile([128, NT * BL], BF16)
            for t in range(NT):
                rows = min(128, BP - t * 128)
                ps = tp_ps.tile([128, 128], BF16, tag="tp")
                nc.tensor.transpose(
                    ps[:rows, :BL], w8[:, t * 128: t * 128 + rows],
                    identb[:BL, :BL])
                nc.vector.tensor_copy(wdiag[:rows, t * BL:(t + 1) * BL],
                                      ps[:rows, :BL])

            # map flat tile t -> (chunk ci, j)
            tmap = []
            for ci, (S, W) in enumerate(CHUNKS):
                for j in range((W + 127) // 128):
                    tmap.append((ci, j))
            ctx_sb = ctxp.tile([BL, E], F32)
            for ec in range(4):
                cps = ctxps.tile([BL, 512], F32, tag="cps")
                for t in range(NT):
                    rows = min(128, BP - t * 128)
                    ci, j = tmap[t]
                    nc.tensor.matmul(
                        cps[:],
                        wdiag[:rows, t * BL:(t + 1) * BL],
                        nats[ci][:rows, j * E + ec * 512: j * E + (ec + 1) * 512],
                        start=(t == 0), stop=(t == NT - 1))
                nc.vector.tensor_copy(ctx_sb[:, ec * 512:(ec + 1) * 512], cps[:])
            nc.sync.dma_start(out_ctx.ap(), ctx_sb[:])

    nc.compile()
    return nc


_NC = None


def _get_nc():
    global _NC
    if _NC is None:
        _NC = build_nc()
    return _NC


def kernel(enc_hiddens, dec_prev_hidden, W1_w, W1_b, W2_w, W2_b, V_w, V_b):
    from concourse import bass_utils

    nc = _get_nc()
    enc_hiddens = np.asarray(enc_hiddens, np.float32)
    dec_prev_hidden = np.asarray(dec_prev_hidden, np.float32)
    shared = {
        "W1_w": np.ascontiguousarray(W1_w, np.float32),
        "W1_b": np.ascontiguousarray(W1_b, np.float32),
        "W2_w": np.ascontiguousarray(W2_w, np.float32),
        "W2_b": np.ascontiguousarray(W2_b, np.float32),
        "V_w": np.ascontiguousarray(V_w, np.float32),
    }
    in_maps = []
    for i in range(NCORES):
        m = dict(shared)
        m["enc_hiddens"] = np.ascontiguousarray(enc_hiddens[i * BL:(i + 1) * BL])
        m["dec_prev_hidden"] = np.ascontiguousarray(
            dec_prev_hidden[i * BL:(i + 1) * BL])
        in_maps.append(m)

    res = bass_utils.run_bass_kernel_spmd(nc, in_maps,
                                          core_ids=list(range(NCORES)))
    outs = res.results
    context = np.concatenate([o["out_ctx"] for o in outs], axis=0).reshape(B, 1, E)
    attn = np.concatenate([o["out_attn"] for o in outs], axis=0)
    return context, attn


# revision 22
# speedup vs baseline: 1.0784x; 1.0784x over previous
"""Bahdanau-attention kernel for Trainium2, data-parallel over batch on 8 NeuronCores.

Per-core shard: 8 batches (1568 flat encoder rows). Design:
  - enc is read from HBM exactly once: SWDGE cast-DMA (f32->bf16) into
    resident SBUF "nat" tiles (one per chunk). PE-transposes produce encT
    (E on partitions) interleaved with the main matmuls so the PE stays warm.
  - W1/W2 are cast to bf16 DRAM staging slices, then loaded transposed via
    the xbar DMA-transpose (few, large instructions; no PE time).
  - Main loop per flat-row chunk [384,384,384,416]: bf16 matmuls accumulate
    enc_projT in f32 PSUM -> ScalarE tanh with per-partition bias
    Hb = h_projT + W1_b + W2_b -> V-dot matmul accumulates scores (f32).
  - Per completed batch: f32 softmax on its scores row; attn row DMA'd out;
    w row cast/scattered into the block-diagonal W8 [8 x 1568].
  - End: W8 column blocks are PE-transposed to wdiag [128 x 8] chunks; the
    context for all 8 batches accumulates in one [8, 512] PSUM per e-chunk
    from the resident nat tiles (enc never re-read).
"""

import sys

sys.path.insert(0, "/opt/trn_rl_repo")
sys.path.insert(0, "/opt/pypackages")

import numpy as np
from contextlib import ExitStack

import concourse.bass as bass
import concourse.bacc as bacc
import concourse.mybir as mybir
import concourse.tile as tile
from concourse.masks import make_identity

F32 = mybir.dt.float32
BF16 = mybir.dt.bfloat16
AF = mybir.ActivationFunctionType
ALU = mybir.AluOpType
AX = mybir.AxisListType

B, P, E, A = 64, 196, 2048, 1024
NCORES = 8
BL = B // NCORES          # 8 local batches per core
BP = BL * P               # 1568 flat rows per core
KE = E // 128             # 16 contraction chunks
KA = A // 128             # 8 attn-dim tiles
CHUNKS = [(0, 384), (384, 384), (768, 384), (1152, 416)]
CW = 416                  # max chunk width
NT = (BP + 127) // 128    # 13 flat 128-row tiles


def _segments(S, W):
    segs = []
    b = S // P
    pos = S
    while pos < S + W:
        end = min((b + 1) * P, S + W)
        segs.append((b, pos - S, end - pos))
        pos = end
        b += 1
    return segs


def _done_batches():
    done = []
    prev = 0
    for S, W in CHUNKS:
        nb = (S + W) // P
        done.append(list(range(prev, nb)))
        prev = nb
    return done


DONE = _done_batches()


def build_nc():
    nc = bacc.Bacc("TRN2", target_bir_lowering=False, debug=False,
                   enable_asserts=False)

    enc = nc.dram_tensor("enc_hiddens", [BL, P, E], F32, kind="ExternalInput")
    dec = nc.dram_tensor("dec_prev_hidden", [BL, E], F32, kind="ExternalInput")
    w1 = nc.dram_tensor("W1_w", [A, E], F32, kind="ExternalInput")
    w1b = nc.dram_tensor("W1_b", [A], F32, kind="ExternalInput")
    w2 = nc.dram_tensor("W2_w", [A, E], F32, kind="ExternalInput")
    w2b = nc.dram_tensor("W2_b", [A], F32, kind="ExternalInput")
    vw = nc.dram_tensor("V_w", [1, A], F32, kind="ExternalInput")
    out_ctx = nc.dram_tensor("out_ctx", [BL, E], F32, kind="ExternalOutput")
    out_attn = nc.dram_tensor("out_attn", [BL, P], F32, kind="ExternalOutput")

    enc_flat = enc.ap().rearrange("b p e -> (b p) e")

    with tile.TileContext(nc) as tc:
        with ExitStack() as ctx:
            const = ctx.enter_context(tc.tile_pool(name="const", bufs=1))
            identb = const.tile([128, 128], BF16)
            make_identity(nc, identb[:])

            # Persistent SBUF tensors
            w2t = const.tile([128, KE * A], BF16)       # W2^T: [e, k-major a]
            hb = const.tile([128, KA * BL], F32)        # Hb[q, a*BL+b]
            vt = const.tile([128, KA], BF16)            # V^T columns per a-tile
            scores = const.tile([1, BP], F32)
            w8 = const.tile([BL, BP], BF16)             # block-diag softmax w
            nc.vector.memset(w8[:], 0.0)
            # resident bf16 enc rows, one tile per chunk: [128, nj*E]
            nats = []
            for ci, (S, W) in enumerate(CHUNKS):
                nj = (W + 127) // 128
                nats.append(const.tile([128, nj * E], BF16, name=f"nat{ci}"))

            tp_ps = ctx.enter_context(
                tc.tile_pool(name="tp_ps", bufs=2, space="PSUM"))

            dramp = ctx.enter_context(
                tc.tile_pool(name="dram", bufs=1, space="DRAM"))
            w2stage = [dramp.tile([A, 512], BF16, tag=f"w2s{q}", name=f"w2stage{q}")
                       for q in range(4)]
            w1stage = [dramp.tile([A, 512], BF16, tag=f"w1s{q}", name=f"w1stage{q}")
                       for q in range(4)]

            # ---- cast-DMA issue order (one SWDGE queue, roughly serial) ----
            def cast_chunk(ci):
                S, W = CHUNKS[ci]
                nfull = W // 128
                nc.gpsimd.dma_start(
                    nats[ci][:].rearrange("p (j e) -> p j e", e=E)[:, :nfull, :],
                    enc_flat[S:S + nfull * 128, :]
                    .rearrange("(j p) e -> p j e", p=128))
                rem = W - nfull * 128
                if rem:
                    nc.gpsimd.dma_start(
                        nats[ci][:rem, nfull * E:nfull * E + E],
                        enc_flat[S + nfull * 128:S + W, :])

            dec_sb = const.tile([BL, E], BF16)
            nc.gpsimd.dma_start(dec_sb[:], dec.ap())
            with nc.allow_non_contiguous_dma(reason="tiny transposed load"):
                nc.gpsimd.dma_start(
                    vt[:], vw.ap().rearrange("o (a k) -> k (o a)", k=128))
            cast_chunk(0)
            for q in range(4):
                nc.gpsimd.dma_start(w2stage[q][:],
                                    w2.ap()[:, q * 512:(q + 1) * 512])
            cast_chunk(1)
            for q in range(4):
                nc.gpsimd.dma_start(w1stage[q][:],
                                    w1.ap()[:, q * 512:(q + 1) * 512])
            cast_chunk(2)
            cast_chunk(3)

            # enc transpose emission helper; chunks 0/1 are emitted before
            # h_proj so the PE has work while the weights stream in
            enctp = ctx.enter_context(tc.tile_pool(name="enct", bufs=2))
            encts = {}

            def do_transposes(ci):
                S, W = CHUNKS[ci]
                nat = nats[ci]
                enct = enctp.tile([128, KE * CW], tag="enct", name=f"enct{ci}",
                                  dtype=BF16)
                encts[ci] = enct
                nj = (W + 127) // 128
                for j in range(nj):
                    rows = min(128, W - j * 128)
                    for k in range(KE):
                        ps = tp_ps.tile([128, 128], BF16, tag="tp",
                                        name=f"tps{ci}_{j}_{k}")
                        nc.tensor.transpose(
                            ps[:, :rows],
                            nat[:rows, j * E + k * 128: j * E + (k + 1) * 128],
                            identb[:rows, :rows])
                        nc.vector.tensor_copy(
                            enct[:, k * CW + j * 128: k * CW + j * 128 + rows],
                            ps[:, :rows])

            do_transposes(0)
            do_transposes(1)

            # ---------------- setup ----------------
            with ExitStack() as sctx:
                sp1 = sctx.enter_context(tc.tile_pool(name="setup1", bufs=1))
                hps_pool = sctx.enter_context(
                    tc.tile_pool(name="hps", bufs=2, space="PSUM"))

                # W1T / W2T via DRAM->SBUF xbar DMA transpose (W1 first: Hb
                # is needed at the first tanh).
                w2t3 = w2t[:].rearrange("p (k a) -> p k a", k=KE)
                w1t = sp1.tile([128, KE * A], BF16)
                w1t3 = w1t[:].rearrange("p (k a) -> p k a", k=KE)
                for q in range(4):
                    nc.scalar.dma_start_transpose(
                        w2t3[:, 4 * q: 4 * (q + 1), :], w2stage[q][:])
                for q in range(4):
                    nc.scalar.dma_start_transpose(
                        w1t3[:, 4 * q: 4 * (q + 1), :], w1stage[q][:])

                # dec -> decT (bf16)
                dect = sp1.tile([128, KE * BL], BF16)
                for k in range(KE):
                    ps = tp_ps.tile([128, 128], BF16, tag="tp")
                    nc.tensor.transpose(
                        ps[:, :BL], dec_sb[:, k * 128:(k + 1) * 128],
                        identb[:BL, :BL])
                    nc.any.tensor_copy(dect[:, k * BL:(k + 1) * BL], ps[:, :BL])

                # h_proj = dec @ W1.T  -> [BL, A] f32
                h_sb = sp1.tile([BL, A], F32)
                for half in range(2):
                    hps = hps_pool.tile([BL, 512], F32, tag="hps")
                    for k in range(KE):
                        nc.tensor.matmul(
                            hps[:],
                            dect[:, k * BL:(k + 1) * BL],
                            w1t[:, k * A + half * 512: k * A + half * 512 + 512],
                            start=(k == 0), stop=(k == KE - 1))
                    nc.any.tensor_copy(h_sb[:, half * 512:(half + 1) * 512],
                                       hps[:])

                # bias columns: W1_b + W2_b, laid out [q, a]  (f32)
                w1bc = sp1.tile([128, KA], F32)
                w2bc = sp1.tile([128, KA], F32)
                with nc.allow_non_contiguous_dma(reason="tiny transposed loads"):
                    nc.sync.dma_start(
                        w1bc[:], w1b.ap().rearrange("(a k) -> k a", k=128))
                    nc.sync.dma_start(
                        w2bc[:], w2b.ap().rearrange("(a k) -> k a", k=128))
                nc.vector.tensor_add(w1bc[:], w1bc[:], w2bc[:])

                # hT + bias -> Hb  (bf16 transpose of h_sb via PE)
                h_bf = sp1.tile([BL, A], BF16)
                nc.vector.tensor_copy(h_bf[:], h_sb[:])
                for a in range(KA):
                    ps = tp_ps.tile([128, 128], BF16, tag="tp")
                    nc.tensor.transpose(
                        ps[:, :BL], h_bf[:, a * 128:(a + 1) * 128],
                        identb[:BL, :BL])
                    nc.vector.tensor_scalar(
                        out=hb[:, a * BL:(a + 1) * BL], in0=ps[:, :BL],
                        scalar1=w1bc[:, a:a + 1], scalar2=None, op0=ALU.add)

            # ---------------- main loop ----------------
            tpool = ctx.enter_context(tc.tile_pool(name="tpool", bufs=3))
            smp = ctx.enter_context(tc.tile_pool(name="smp", bufs=2))
            bigps = ctx.enter_context(
                tc.tile_pool(name="bigps", bufs=3, space="PSUM"))
            scps = ctx.enter_context(
                tc.tile_pool(name="scps", bufs=1, space="PSUM"))

            for ci, (S, W) in enumerate(CHUNKS):
                if ci not in encts:
                    do_transposes(ci)
                enct = encts[ci]
                segs = _segments(S, W)
                sc = scps.tile([1, CW], F32, tag="sc")
                for a in range(KA):
                    ps = bigps.tile([128, CW], F32, tag="big")
                    for k in range(KE):
                        nc.tensor.matmul(
                            ps[:, :W],
                            w2t[:, k * A + a * 128: k * A + (a + 1) * 128],
                            enct[:, k * CW: k * CW + W],
                            start=(k == 0), stop=(k == KE - 1))
                    t_sb = tpool.tile([128, CW], BF16, tag="t")
                    for (b, off, ln) in segs:
                        nc.scalar.activation(
                            t_sb[:, off:off + ln], ps[:, off:off + ln], AF.Tanh,
                            bias=hb[:, a * BL + b: a * BL + b + 1])
                    nc.tensor.matmul(
                        sc[:, :W], vt[:, a:a + 1], t_sb[:, :W],
                        start=(a == 0), stop=(a == KA - 1))
                nc.any.tensor_copy(scores[:, S:S + W], sc[:, :W])

                for b in DONE[ci]:
                    sseg = scores[:, b * P:(b + 1) * P]
                    negmax = smp.tile([1, 1], F32, tag="nm")
                    nc.vector.reduce_max(negmax[:], sseg, axis=AX.X, negate=True)
                    ex = smp.tile([1, P], F32, tag="ex")
                    nc.scalar.activation(ex[:], sseg, AF.Exp, bias=negmax[:])
                    ssum = smp.tile([1, 1], F32, tag="sm")
                    nc.vector.reduce_sum(ssum[:], ex[:], axis=AX.X)
                    rcp = smp.tile([1, 1], F32, tag="rc")
                    nc.vector.reciprocal(rcp[:], ssum[:])
                    wsb = smp.tile([1, P], F32, tag="w")
                    nc.vector.tensor_scalar(
                        out=wsb[:], in0=ex[:], scalar1=rcp[:], scalar2=None,
                        op0=ALU.mult)
                    nc.sync.dma_start(out_attn.ap()[b:b + 1, :], wsb[:])
                    # scatter w row (cast to bf16) onto the block diagonal
                    nc.gpsimd.dma_start(
                        w8[b:b + 1, b * P:(b + 1) * P], wsb[:])

            # ---------------- context (end phase) ----------------
            ctxps = ctx.enter_context(
                tc.tile_pool(name="ctxps", bufs=2, space="PSUM"))
            ctxp = ctx.enter_context(tc.tile_pool(name="ctxsb", bufs=1))
            wdiag = const.tile([128, NT * BL], BF16)
            for t in range(NT):
                rows = min(128, BP - t * 128)
                ps = tp_ps.tile([128, 128], BF16, tag="tp")
                nc.tensor.transpose(
                    ps[:rows, :BL], w8[:, t * 128: t * 128 + rows],
                    identb[:BL, :BL])
                nc.vector.tensor_copy(wdiag[:rows, t * BL:(t + 1) * BL],
                                      ps[:rows, :BL])

            # map flat tile t -> (chunk ci, j)
            tmap = []
            for ci, (S, W) in enumerate(CHUNKS):
                for j in range((W + 127) // 128):
                    tmap.append((ci, j))
            ctx_sb = ctxp.tile([BL, E], F32)
            for ec in range(4):
                cps = ctxps.tile([BL, 512], F32, tag="cps")
                for t in range(NT):
                    rows = min(128, BP - t * 128)
                    ci, j = tmap[t]
                    nc.tensor.matmul(
                        cps[:],
                        wdiag[:rows, t * BL:(t + 1) * BL],
                        nats[ci][:rows, j * E + ec * 512: j * E + (ec + 1) * 512],
                        start=(t == 0), stop=(t == NT - 1))
                nc.vector.tensor_copy(ctx_sb[:, ec * 512:(ec + 1) * 512], cps[:])
            nc.sync.dma_start(out_ctx.ap(), ctx_sb[:])

    nc.compile()
    return nc


_NC = None


def _get_nc():
    global _NC
    if _NC is None:
        _NC = build_nc()
    return _NC


def kernel(enc_hiddens, dec_prev_hidden, W1_w, W1_b, W2_w, W2_b, V_w, V_b):
    from concourse import bass_utils

    nc = _get_nc()
    enc_hiddens = np.asarray(enc_hiddens, np.float32)
    dec_prev_hidden = np.asarray(dec_prev_hidden, np.float32)
    shared = {
        "W1_w": np.ascontiguousarray(W1_w, np.float32),
        "W1_b": np.ascontiguousarray(W1_b, np.float32),
        "W2_w": np.ascontiguousarray(W2_w, np.float32),
        "W2_b": np.ascontiguousarray(W2_b, np.float32),
        "V_w": np.ascontiguousarray(V_w, np.float32),
    }
    in_maps = []
    for i in range(NCORES):
        m = dict(shared)
        m["enc_hiddens"] = np.ascontiguousarray(enc_hiddens[i * BL:(i + 1) * BL])
        m["dec_prev_hidden"] = np.ascontiguousarray(
            dec_prev_hidden[i * BL:(i + 1) * BL])
        in_maps.append(m)

    res = bass_utils.run_bass_kernel_spmd(nc, in_maps,
                                          core_ids=list(range(NCORES)))
    outs = res.results
    context = np.concatenate([o["out_ctx"] for o in outs], axis=0).reshape(B, 1, E)
    attn = np.concatenate([o["out_attn"] for o in outs], axis=0)
    return context, attn


# revision 23
# speedup vs baseline: 1.0966x; 1.0169x over previous
"""Bahdanau-attention kernel for Trainium2, data-parallel over batch on 8 NeuronCores.

Per-core shard: 8 batches (1568 flat encoder rows). Design:
  - enc is read from HBM exactly once: SWDGE cast-DMA (f32->bf16) into
    resident SBUF "nat" tiles (one per chunk). PE-transposes produce encT
    (E on partitions) interleaved with the main matmuls so the PE stays warm.
  - W1/W2 are cast to bf16 DRAM staging slices, then loaded transposed via
    the xbar DMA-transpose (few, large instructions; no PE time).
  - Main loop per flat-row chunk [384,384,384,416]: bf16 matmuls accumulate
    enc_projT in f32 PSUM -> ScalarE tanh with per-partition bias
    Hb = h_projT + W1_b + W2_b -> V-dot matmul accumulates scores (f32).
  - Per completed batch: f32 softmax on its scores row; attn row DMA'd out;
    w row cast/scattered into the block-diagonal W8 [8 x 1568].
  - End: W8 column blocks are PE-transposed to wdiag [128 x 8] chunks; the
    context for all 8 batches accumulates in one [8, 512] PSUM per e-chunk
    from the resident nat tiles (enc never re-read).
"""

import sys

sys.path.insert(0, "/opt/trn_rl_repo")
sys.path.insert(0, "/opt/pypackages")

import numpy as np
from contextlib import ExitStack

import concourse.bass as bass
import concourse.bacc as bacc
import concourse.mybir as mybir
import concourse.tile as tile
from concourse.masks import make_identity

F32 = mybir.dt.float32
BF16 = mybir.dt.bfloat16
AF = mybir.ActivationFunctionType
ALU = mybir.AluOpType
AX = mybir.AxisListType

B, P, E, A = 64, 196, 2048, 1024
NCORES = 8
BL = B // NCORES          # 8 local batches per core
BP = BL * P               # 1568 flat rows per core
KE = E // 128             # 16 contraction chunks
KA = A // 128             # 8 attn-dim tiles
CHUNKS = [(0, 384), (384, 384), (768, 384), (1152, 416)]
CW = 416                  # max chunk width
NT = (BP + 127) // 128    # 13 flat 128-row tiles


def _segments(S, W):
    segs = []
    b = S // P
    pos = S
    while pos < S + W:
        end = min((b + 1) * P, S + W)
        segs.append((b, pos - S, end - pos))
        pos = end
        b += 1
    return segs


def _done_batches():
    done = []
    prev = 0
    for S, W in CHUNKS:
        nb = (S + W) // P
        done.append(list(range(prev, nb)))
        prev = nb
    return done


DONE = _done_batches()


def build_nc():
    nc = bacc.Bacc("TRN2", target_bir_lowering=False, debug=False,
                   enable_asserts=False)

    enc = nc.dram_tensor("enc_hiddens", [BL, P, E], F32, kind="ExternalInput")
    dec = nc.dram_tensor("dec_prev_hidden", [BL, E], F32, kind="ExternalInput")
    w1 = nc.dram_tensor("W1_w", [A, E], F32, kind="ExternalInput")
    w1b = nc.dram_tensor("W1_b", [A], F32, kind="ExternalInput")
    w2 = nc.dram_tensor("W2_w", [A, E], F32, kind="ExternalInput")
    w2b = nc.dram_tensor("W2_b", [A], F32, kind="ExternalInput")
    vw = nc.dram_tensor("V_w", [1, A], F32, kind="ExternalInput")
    out_ctx = nc.dram_tensor("out_ctx", [BL, E], F32, kind="ExternalOutput")
    out_attn = nc.dram_tensor("out_attn", [BL, P], F32, kind="ExternalOutput")

    enc_flat = enc.ap().rearrange("b p e -> (b p) e")

    with tile.TileContext(nc) as tc:
        with ExitStack() as ctx:
            const = ctx.enter_context(tc.tile_pool(name="const", bufs=1))
            identb = const.tile([128, 128], BF16)
            make_identity(nc, identb[:])

            # Persistent SBUF tensors
            w2t = const.tile([128, KE * A], BF16)       # W2^T: [e, k-major a]
            hb = const.tile([128, KA * BL], F32)        # Hb[q, a*BL+b]
            vt = const.tile([128, KA], BF16)            # V^T columns per a-tile
            scores = const.tile([1, BP], F32)
            w8 = const.tile([BL, BP], BF16)             # block-diag softmax w
            nc.vector.memset(w8[:], 0.0)
            # resident bf16 enc rows, one tile per chunk: [128, nj*E]
            nats = []
            for ci, (S, W) in enumerate(CHUNKS):
                nj = (W + 127) // 128
                nats.append(const.tile([128, nj * E], BF16, name=f"nat{ci}"))

            tp_ps = ctx.enter_context(
                tc.tile_pool(name="tp_ps", bufs=2, space="PSUM"))

            dramp = ctx.enter_context(
                tc.tile_pool(name="dram", bufs=1, space="DRAM"))
            w2stage = dramp.tile([A, E], BF16, tag="w2s", name="w2stage")
            w1stage = dramp.tile([A, E], BF16, tag="w1s", name="w1stage")

            # ---- cast-DMA issue order (one SWDGE queue, roughly serial) ----
            def cast_chunk(ci):
                S, W = CHUNKS[ci]
                nfull = W // 128
                nc.gpsimd.dma_start(
                    nats[ci][:].rearrange("p (j e) -> p j e", e=E)[:, :nfull, :],
                    enc_flat[S:S + nfull * 128, :]
                    .rearrange("(j p) e -> p j e", p=128))
                rem = W - nfull * 128
                if rem:
                    nc.gpsimd.dma_start(
                        nats[ci][:rem, nfull * E:nfull * E + E],
                        enc_flat[S + nfull * 128:S + W, :])

            dec_sb = const.tile([BL, E], BF16)
            nc.gpsimd.dma_start(dec_sb[:], dec.ap())
            with nc.allow_non_contiguous_dma(reason="tiny transposed load"):
                nc.gpsimd.dma_start(
                    vt[:], vw.ap().rearrange("o (a k) -> k (o a)", k=128))
            cast_chunk(0)
            nc.gpsimd.dma_start(w1stage[:], w1.ap())
            cast_chunk(1)
            nc.gpsimd.dma_start(w2stage[:], w2.ap())
            cast_chunk(2)
            cast_chunk(3)

            # enc transpose emission helper; chunks 0/1 are emitted before
            # h_proj so the PE has work while the weights stream in
            enctp = ctx.enter_context(tc.tile_pool(name="enct", bufs=2))
            encts = {}

            def do_transposes(ci):
                S, W = CHUNKS[ci]
                nat = nats[ci]
                enct = enctp.tile([128, KE * CW], tag="enct", name=f"enct{ci}",
                                  dtype=BF16)
                encts[ci] = enct
                nj = (W + 127) // 128
                for j in range(nj):
                    rows = min(128, W - j * 128)
                    for k in range(KE):
                        ps = tp_ps.tile([128, 128], BF16, tag="tp",
                                        name=f"tps{ci}_{j}_{k}")
                        nc.tensor.transpose(
                            ps[:, :rows],
                            nat[:rows, j * E + k * 128: j * E + (k + 1) * 128],
                            identb[:rows, :rows])
                        nc.vector.tensor_copy(
                            enct[:, k * CW + j * 128: k * CW + j * 128 + rows],
                            ps[:, :rows])

            do_transposes(0)
            do_transposes(1)

            # ---------------- setup ----------------
            with ExitStack() as sctx:
                sp1 = sctx.enter_context(tc.tile_pool(name="setup1", bufs=1))
                hps_pool = sctx.enter_context(
                    tc.tile_pool(name="hps", bufs=2, space="PSUM"))

                # W1T / W2T via DRAM->SBUF xbar DMA transpose (W1 first: Hb
                # is needed at the first tanh).
                w2t3 = w2t[:].rearrange("p (k a) -> p k a", k=KE)
                w1t = sp1.tile([128, KE * A], BF16)
                w1t3 = w1t[:].rearrange("p (k a) -> p k a", k=KE)
                for q in range(4):
                    nc.sync.dma_start_transpose(
                        w1t3[:, 4 * q: 4 * (q + 1), :],
                        w1stage[:, q * 512:(q + 1) * 512])
                for q in range(4):
                    nc.sync.dma_start_transpose(
                        w2t3[:, 4 * q: 4 * (q + 1), :],
                        w2stage[:, q * 512:(q + 1) * 512])

                # dec -> decT (bf16)
                dect = sp1.tile([128, KE * BL], BF16)
                for k in range(KE):
                    ps = tp_ps.tile([128, 128], BF16, tag="tp")
                    nc.tensor.transpose(
                        ps[:, :BL], dec_sb[:, k * 128:(k + 1) * 128],
                        identb[:BL, :BL])
                    nc.any.tensor_copy(dect[:, k * BL:(k + 1) * BL], ps[:, :BL])

                # h_proj = dec @ W1.T  -> [BL, A] f32
                h_sb = sp1.tile([BL, A], F32)
                for half in range(2):
                    hps = hps_pool.tile([BL, 512], F32, tag="hps")
                    for k in range(KE):
                        nc.tensor.matmul(
                            hps[:],
                            dect[:, k * BL:(k + 1) * BL],
                            w1t[:, k * A + half * 512: k * A + half * 512 + 512],
                            start=(k == 0), stop=(k == KE - 1))
                    nc.any.tensor_copy(h_sb[:, half * 512:(half + 1) * 512],
                                       hps[:])

                # bias columns: W1_b + W2_b, laid out [q, a]  (f32)
                w1bc = sp1.tile([128, KA], F32)
                w2bc = sp1.tile([128, KA], F32)
                with nc.allow_non_contiguous_dma(reason="tiny transposed loads"):
                    nc.sync.dma_start(
                        w1bc[:], w1b.ap().rearrange("(a k) -> k a", k=128))
                    nc.sync.dma_start(
                        w2bc[:], w2b.ap().rearrange("(a k) -> k a", k=128))
                nc.vector.tensor_add(w1bc[:], w1bc[:], w2bc[:])

                # hT + bias -> Hb  (bf16 transpose of h_sb via PE)
                h_bf = sp1.tile([BL, A], BF16)
                nc.vector.tensor_copy(h_bf[:], h_sb[:])
                for a in range(KA):
                    ps = tp_ps.tile([128, 128], BF16, tag="tp")
                    nc.tensor.transpose(
                        ps[:, :BL], h_bf[:, a * 128:(a + 1) * 128],
                        identb[:BL, :BL])
                    nc.vector.tensor_scalar(
                        out=hb[:, a * BL:(a + 1) * BL], in0=ps[:, :BL],
                        scalar1=w1bc[:, a:a + 1], scalar2=None, op0=ALU.add)

            # ---------------- main loop ----------------
            tpool = ctx.enter_context(tc.tile_pool(name="tpool", bufs=3))
            smp = ctx.enter_context(tc.tile_pool(name="smp", bufs=2))
            bigps = ctx.enter_context(
                tc.tile_pool(name="bigps", bufs=3, space="PSUM"))
            scps = ctx.enter_context(
                tc.tile_pool(name="scps", bufs=1, space="PSUM"))

            for ci, (S, W) in enumerate(CHUNKS):
                if ci not in encts:
                    do_transposes(ci)
                enct = encts[ci]
                segs = _segments(S, W)
                sc = scps.tile([1, CW], F32, tag="sc")
                for a in range(KA):
                    ps = bigps.tile([128, CW], F32, tag="big")
                    for k in range(KE):
                        nc.tensor.matmul(
                            ps[:, :W],
                            w2t[:, k * A + a * 128: k * A + (a + 1) * 128],
                            enct[:, k * CW: k * CW + W],
                            start=(k == 0), stop=(k == KE - 1))
                    t_sb = tpool.tile([128, CW], BF16, tag="t")
                    for (b, off, ln) in segs:
                        nc.scalar.activation(
                            t_sb[:, off:off + ln], ps[:, off:off + ln], AF.Tanh,
                            bias=hb[:, a * BL + b: a * BL + b + 1])
                    nc.tensor.matmul(
                        sc[:, :W], vt[:, a:a + 1], t_sb[:, :W],
                        start=(a == 0), stop=(a == KA - 1))
                nc.any.tensor_copy(scores[:, S:S + W], sc[:, :W])

                for b in DONE[ci]:
                    sseg = scores[:, b * P:(b + 1) * P]
                    negmax = smp.tile([1, 1], F32, tag="nm")
                    nc.vector.reduce_max(negmax[:], sseg, axis=AX.X, negate=True)
                    ex = smp.tile([1, P], F32, tag="ex")
                    nc.scalar.activation(ex[:], sseg, AF.Exp, bias=negmax[:])
                    ssum = smp.tile([1, 1], F32, tag="sm")
                    nc.vector.reduce_sum(ssum[:], ex[:], axis=AX.X)
                    rcp = smp.tile([1, 1], F32, tag="rc")
                    nc.vector.reciprocal(rcp[:], ssum[:])
                    wsb = smp.tile([1, P], F32, tag="w")
                    nc.vector.tensor_scalar(
                        out=wsb[:], in0=ex[:], scalar1=rcp[:], scalar2=None,
                        op0=ALU.mult)
                    nc.sync.dma_start(out_attn.ap()[b:b + 1, :], wsb[:])
                    # scatter w row (cast to bf16) onto the block diagonal
                    nc.gpsimd.dma_start(
                        w8[b:b + 1, b * P:(b + 1) * P], wsb[:])

            # ---------------- context (end phase) ----------------
            ctxps = ctx.enter_context(
                tc.tile_pool(name="ctxps", bufs=2, space="PSUM"))
            ctxp = ctx.enter_context(tc.tile_pool(name="ctxsb", bufs=1))
            wdiag = const.tile([128, NT * BL], BF16)
            for t in range(NT):
                rows = min(128, BP - t * 128)
                ps = tp_ps.tile([128, 128], BF16, tag="tp")
                nc.tensor.transpose(
                    ps[:rows, :BL], w8[:, t * 128: t * 128 + rows],
                    identb[:BL, :BL])
                nc.vector.tensor_copy(wdiag[:rows, t * BL:(t + 1) * BL],
                                      ps[:rows, :BL])

            # map flat tile t -> (chunk ci, j)
            tmap = []
            for ci, (S, W) in enumerate(CHUNKS):
                for j in range((W + 127) // 128):
                    tmap.append((ci, j))
            ctx_sb = ctxp.tile([BL, E], F32)
            for ec in range(4):
                cps = ctxps.tile([BL, 512], F32, tag="cps")
                for t in range(NT):
                    rows = min(128, BP - t * 128)
                    ci, j = tmap[t]
                    nc.tensor.matmul(
                        cps[:],
                        wdiag[:rows, t * BL:(t + 1) * BL],
                        nats[ci][:rows, j * E + ec * 512: j * E + (ec + 1) * 512],
                        start=(t == 0), stop=(t == NT - 1))
                nc.vector.tensor_copy(ctx_sb[:, ec * 512:(ec + 1) * 512], cps[:])
            nc.sync.dma_start(out_ctx.ap(), ctx_sb[:])

    nc.compile()
    return nc


_NC = None


def _get_nc():
    global _NC
    if _NC is None:
        _NC = build_nc()
    return _NC


def kernel(enc_hiddens, dec_prev_hidden, W1_w, W1_b, W2_w, W2_b, V_w, V_b):
    from concourse import bass_utils

    nc = _get_nc()
    enc_hiddens = np.asarray(enc_hiddens, np.float32)
    dec_prev_hidden = np.asarray(dec_prev_hidden, np.float32)
    shared = {
        "W1_w": np.ascontiguousarray(W1_w, np.float32),
        "W1_b": np.ascontiguousarray(W1_b, np.float32),
        "W2_w": np.ascontiguousarray(W2_w, np.float32),
        "W2_b": np.ascontiguousarray(W2_b, np.float32),
        "V_w": np.ascontiguousarray(V_w, np.float32),
    }
    in_maps = []
    for i in range(NCORES):
        m = dict(shared)
        m["enc_hiddens"] = np.ascontiguousarray(enc_hiddens[i * BL:(i + 1) * BL])
        m["dec_prev_hidden"] = np.ascontiguousarray(
            dec_prev_hidden[i * BL:(i + 1) * BL])
        in_maps.append(m)

    res = bass_utils.run_bass_kernel_spmd(nc, in_maps,
                                          core_ids=list(range(NCORES)))
    outs = res.results
    context = np.concatenate([o["out_ctx"] for o in outs], axis=0).reshape(B, 1, E)
    attn = np.concatenate([o["out_attn"] for o in outs], axis=0)
    return context, attn


# revision 24
# speedup vs baseline: 1.1107x; 1.0129x over previous
"""Bahdanau-attention kernel for Trainium2, data-parallel over batch on 8 NeuronCores.

Per-core shard: 8 batches (1568 flat encoder rows). Design:
  - enc is read from HBM exactly once: SWDGE cast-DMA (f32->bf16) into
    resident SBUF "nat" tiles (one per chunk). PE-transposes produce encT
    (E on partitions) interleaved with the main matmuls so the PE stays warm.
  - W1/W2 are cast to bf16 DRAM staging slices, then loaded transposed via
    the xbar DMA-transpose (few, large instructions; no PE time).
  - Main loop per flat-row chunk [384,384,384,416]: bf16 matmuls accumulate
    enc_projT in f32 PSUM -> ScalarE tanh with per-partition bias
    Hb = h_projT + W1_b + W2_b -> V-dot matmul accumulates scores (f32).
  - Per completed batch: f32 softmax on its scores row; attn row DMA'd out;
    w row cast/scattered into the block-diagonal W8 [8 x 1568].
  - End: W8 column blocks are PE-transposed to wdiag [128 x 8] chunks; the
    context for all 8 batches accumulates in one [8, 512] PSUM per e-chunk
    from the resident nat tiles (enc never re-read).
"""

import sys

sys.path.insert(0, "/opt/trn_rl_repo")
sys.path.insert(0, "/opt/pypackages")

import numpy as np
from contextlib import ExitStack

import concourse.bass as bass
import concourse.bacc as bacc
import concourse.mybir as mybir
import concourse.tile as tile
from concourse.masks import make_identity

F32 = mybir.dt.float32
BF16 = mybir.dt.bfloat16
AF = mybir.ActivationFunctionType
ALU = mybir.AluOpType
AX = mybir.AxisListType

B, P, E, A = 64, 196, 2048, 1024
NCORES = 8
BL = B // NCORES          # 8 local batches per core
BP = BL * P               # 1568 flat rows per core
KE = E // 128             # 16 contraction chunks
KA = A // 128             # 8 attn-dim tiles
CHUNKS = [(0, 384), (384, 384), (768, 384), (1152, 416)]
CW = 416                  # max chunk width
NT = (BP + 127) // 128    # 13 flat 128-row tiles


def _segments(S, W):
    segs = []
    b = S // P
    pos = S
    while pos < S + W:
        end = min((b + 1) * P, S + W)
        segs.append((b, pos - S, end - pos))
        pos = end
        b += 1
    return segs


def _done_batches():
    done = []
    prev = 0
    for S, W in CHUNKS:
        nb = (S + W) // P
        done.append(list(range(prev, nb)))
        prev = nb
    return done


DONE = _done_batches()


def build_nc():
    nc = bacc.Bacc("TRN2", target_bir_lowering=False, debug=False,
                   enable_asserts=False)

    enc = nc.dram_tensor("enc_hiddens", [BL, P, E], F32, kind="ExternalInput")
    dec = nc.dram_tensor("dec_prev_hidden", [BL, E], F32, kind="ExternalInput")
    w1 = nc.dram_tensor("W1_w", [A, E], F32, kind="ExternalInput")
    w1b = nc.dram_tensor("W1_b", [A], F32, kind="ExternalInput")
    w2 = nc.dram_tensor("W2_w", [A, E], F32, kind="ExternalInput")
    w2b = nc.dram_tensor("W2_b", [A], F32, kind="ExternalInput")
    vw = nc.dram_tensor("V_w", [1, A], F32, kind="ExternalInput")
    out_ctx = nc.dram_tensor("out_ctx", [BL, E], F32, kind="ExternalOutput")
    out_attn = nc.dram_tensor("out_attn", [BL, P], F32, kind="ExternalOutput")

    enc_flat = enc.ap().rearrange("b p e -> (b p) e")

    with tile.TileContext(nc) as tc:
        with ExitStack() as ctx:
            const = ctx.enter_context(tc.tile_pool(name="const", bufs=1))
            identb = const.tile([128, 128], BF16)
            make_identity(nc, identb[:])

            # Persistent SBUF tensors
            w2t = const.tile([128, KE * A], BF16)       # W2^T: [e, k-major a]
            hb = const.tile([128, KA * BL], F32)        # Hb[q, a*BL+b]
            vt = const.tile([128, KA], BF16)            # V^T columns per a-tile
            scores = const.tile([1, BP], F32)
            w8 = const.tile([BL, BP], BF16)             # block-diag softmax w
            nc.vector.memset(w8[:], 0.0)
            # resident bf16 enc rows, one tile per chunk: [128, nj*E]
            nats = []
            for ci, (S, W) in enumerate(CHUNKS):
                nj = (W + 127) // 128
                nats.append(const.tile([128, nj * E], BF16, name=f"nat{ci}"))

            tp_ps = ctx.enter_context(
                tc.tile_pool(name="tp_ps", bufs=2, space="PSUM"))

            dramp = ctx.enter_context(
                tc.tile_pool(name="dram", bufs=1, space="DRAM"))
            w2stage = dramp.tile([A, E], BF16, tag="w2s", name="w2stage")
            w1stage = dramp.tile([A, E], BF16, tag="w1s", name="w1stage")

            # ---- cast-DMA issue order (one SWDGE queue, roughly serial) ----
            def cast_chunk(ci):
                S, W = CHUNKS[ci]
                nfull = W // 128
                nc.gpsimd.dma_start(
                    nats[ci][:].rearrange("p (j e) -> p j e", e=E)[:, :nfull, :],
                    enc_flat[S:S + nfull * 128, :]
                    .rearrange("(j p) e -> p j e", p=128))
                rem = W - nfull * 128
                if rem:
                    nc.gpsimd.dma_start(
                        nats[ci][:rem, nfull * E:nfull * E + E],
                        enc_flat[S + nfull * 128:S + W, :])

            dec_sb = const.tile([BL, E], BF16)
            nc.gpsimd.dma_start(dec_sb[:], dec.ap())
            with nc.allow_non_contiguous_dma(reason="tiny transposed load"):
                nc.gpsimd.dma_start(
                    vt[:], vw.ap().rearrange("o (a k) -> k (o a)", k=128))
            cast_chunk(0)
            nc.gpsimd.dma_start(w2stage[:], w2.ap())
            nc.gpsimd.dma_start(w1stage[:], w1.ap())
            cast_chunk(1)
            cast_chunk(2)
            cast_chunk(3)

            # enc transpose emission helper; chunks 0/1 are emitted before
            # h_proj so the PE has work while the weights stream in
            enctp = ctx.enter_context(tc.tile_pool(name="enct", bufs=2))
            encts = {}

            def do_transposes(ci):
                S, W = CHUNKS[ci]
                nat = nats[ci]
                enct = enctp.tile([128, KE * CW], tag="enct", name=f"enct{ci}",
                                  dtype=BF16)
                encts[ci] = enct
                nj = (W + 127) // 128
                for j in range(nj):
                    rows = min(128, W - j * 128)
                    for k in range(KE):
                        ps = tp_ps.tile([128, 128], BF16, tag="tp",
                                        name=f"tps{ci}_{j}_{k}")
                        nc.tensor.transpose(
                            ps[:, :rows],
                            nat[:rows, j * E + k * 128: j * E + (k + 1) * 128],
                            identb[:rows, :rows])
                        nc.vector.tensor_copy(
                            enct[:, k * CW + j * 128: k * CW + j * 128 + rows],
                            ps[:, :rows])

            do_transposes(0)
            do_transposes(1)

            # ---------------- setup ----------------
            with ExitStack() as sctx:
                sp1 = sctx.enter_context(tc.tile_pool(name="setup1", bufs=1))
                hps_pool = sctx.enter_context(
                    tc.tile_pool(name="hps", bufs=2, space="PSUM"))

                # W1T / W2T via DRAM->SBUF xbar DMA transpose (W1 first: Hb
                # is needed at the first tanh).
                w2t3 = w2t[:].rearrange("p (k a) -> p k a", k=KE)
                w1t = sp1.tile([128, KE * A], BF16)
                w1t3 = w1t[:].rearrange("p (k a) -> p k a", k=KE)
                for q in range(4):
                    nc.sync.dma_start_transpose(
                        w2t3[:, 4 * q: 4 * (q + 1), :],
                        w2stage[:, q * 512:(q + 1) * 512])
                for q in range(4):
                    nc.sync.dma_start_transpose(
                        w1t3[:, 4 * q: 4 * (q + 1), :],
                        w1stage[:, q * 512:(q + 1) * 512])

                # dec -> decT (bf16)
                dect = sp1.tile([128, KE * BL], BF16)
                for k in range(KE):
                    ps = tp_ps.tile([128, 128], BF16, tag="tp")
                    nc.tensor.transpose(
                        ps[:, :BL], dec_sb[:, k * 128:(k + 1) * 128],
                        identb[:BL, :BL])
                    nc.any.tensor_copy(dect[:, k * BL:(k + 1) * BL], ps[:, :BL])

                # h_proj = dec @ W1.T  -> [BL, A] f32
                h_sb = sp1.tile([BL, A], F32)
                for half in range(2):
                    hps = hps_pool.tile([BL, 512], F32, tag="hps")
                    for k in range(KE):
                        nc.tensor.matmul(
                            hps[:],
                            dect[:, k * BL:(k + 1) * BL],
                            w1t[:, k * A + half * 512: k * A + half * 512 + 512],
                            start=(k == 0), stop=(k == KE - 1))
                    nc.any.tensor_copy(h_sb[:, half * 512:(half + 1) * 512],
                                       hps[:])

                # bias columns: W1_b + W2_b, laid out [q, a]  (f32)
                w1bc = sp1.tile([128, KA], F32)
                w2bc = sp1.tile([128, KA], F32)
                with nc.allow_non_contiguous_dma(reason="tiny transposed loads"):
                    nc.sync.dma_start(
                        w1bc[:], w1b.ap().rearrange("(a k) -> k a", k=128))
                    nc.sync.dma_start(
                        w2bc[:], w2b.ap().rearrange("(a k) -> k a", k=128))
                nc.vector.tensor_add(w1bc[:], w1bc[:], w2bc[:])

                # hT + bias -> Hb  (bf16 transpose of h_sb via PE)
                h_bf = sp1.tile([BL, A], BF16)
                nc.vector.tensor_copy(h_bf[:], h_sb[:])
                for a in range(KA):
                    ps = tp_ps.tile([128, 128], BF16, tag="tp")
                    nc.tensor.transpose(
                        ps[:, :BL], h_bf[:, a * 128:(a + 1) * 128],
                        identb[:BL, :BL])
                    nc.vector.tensor_scalar(
                        out=hb[:, a * BL:(a + 1) * BL], in0=ps[:, :BL],
                        scalar1=w1bc[:, a:a + 1], scalar2=None, op0=ALU.add)

            # ---------------- main loop ----------------
            tpool = ctx.enter_context(tc.tile_pool(name="tpool", bufs=3))
            smp = ctx.enter_context(tc.tile_pool(name="smp", bufs=2))
            bigps = ctx.enter_context(
                tc.tile_pool(name="bigps", bufs=3, space="PSUM"))
            scps = ctx.enter_context(
                tc.tile_pool(name="scps", bufs=1, space="PSUM"))

            for ci, (S, W) in enumerate(CHUNKS):
                if ci not in encts:
                    do_transposes(ci)
                enct = encts[ci]
                segs = _segments(S, W)
                sc = scps.tile([1, CW], F32, tag="sc")
                for a in range(KA):
                    ps = bigps.tile([128, CW], F32, tag="big")
                    for k in range(KE):
                        nc.tensor.matmul(
                            ps[:, :W],
                            w2t[:, k * A + a * 128: k * A + (a + 1) * 128],
                            enct[:, k * CW: k * CW + W],
                            start=(k == 0), stop=(k == KE - 1))
                    t_sb = tpool.tile([128, CW], BF16, tag="t")
                    for (b, off, ln) in segs:
                        nc.scalar.activation(
                            t_sb[:, off:off + ln], ps[:, off:off + ln], AF.Tanh,
                            bias=hb[:, a * BL + b: a * BL + b + 1])
                    nc.tensor.matmul(
                        sc[:, :W], vt[:, a:a + 1], t_sb[:, :W],
                        start=(a == 0), stop=(a == KA - 1))
                nc.any.tensor_copy(scores[:, S:S + W], sc[:, :W])

                for b in DONE[ci]:
                    sseg = scores[:, b * P:(b + 1) * P]
                    negmax = smp.tile([1, 1], F32, tag="nm")
                    nc.vector.reduce_max(negmax[:], sseg, axis=AX.X, negate=True)
                    ex = smp.tile([1, P], F32, tag="ex")
                    nc.scalar.activation(ex[:], sseg, AF.Exp, bias=negmax[:])
                    ssum = smp.tile([1, 1], F32, tag="sm")
                    nc.vector.reduce_sum(ssum[:], ex[:], axis=AX.X)
                    rcp = smp.tile([1, 1], F32, tag="rc")
                    nc.vector.reciprocal(rcp[:], ssum[:])
                    wsb = smp.tile([1, P], F32, tag="w")
                    nc.vector.tensor_scalar(
                        out=wsb[:], in0=ex[:], scalar1=rcp[:], scalar2=None,
                        op0=ALU.mult)
                    nc.sync.dma_start(out_attn.ap()[b:b + 1, :], wsb[:])
                    # scatter w row (cast to bf16) onto the block diagonal
                    nc.gpsimd.dma_start(
                        w8[b:b + 1, b * P:(b + 1) * P], wsb[:])

            # ---------------- context (end phase) ----------------
            ctxps = ctx.enter_context(
                tc.tile_pool(name="ctxps", bufs=2, space="PSUM"))
            ctxp = ctx.enter_context(tc.tile_pool(name="ctxsb", bufs=1))
            wdiag = const.tile([128, NT * BL], BF16)
            for t in range(NT):
                rows = min(128, BP - t * 128)
                ps = tp_ps.tile([128, 128], BF16, tag="tp")
                nc.tensor.transpose(
                    ps[:rows, :BL], w8[:, t * 128: t * 128 + rows],
                    identb[:BL, :BL])
                nc.vector.tensor_copy(wdiag[:rows, t * BL:(t + 1) * BL],
                                      ps[:rows, :BL])

            # map flat tile t -> (chunk ci, j)
            tmap = []
            for ci, (S, W) in enumerate(CHUNKS):
                for j in range((W + 127) // 128):
                    tmap.append((ci, j))
            ctx_sb = ctxp.tile([BL, E], F32)
            for ec in range(4):
                cps = ctxps.tile([BL, 512], F32, tag="cps")
                for t in range(NT):
                    rows = min(128, BP - t * 128)
                    ci, j = tmap[t]
                    nc.tensor.matmul(
                        cps[:],
                        wdiag[:rows, t * BL:(t + 1) * BL],
                        nats[ci][:rows, j * E + ec * 512: j * E + (ec + 1) * 512],
                        start=(t == 0), stop=(t == NT - 1))
                nc.vector.tensor_copy(ctx_sb[:, ec * 512:(ec + 1) * 512], cps[:])
            nc.sync.dma_start(out_ctx.ap(), ctx_sb[:])

    nc.compile()
    return nc


_NC = None


def _get_nc():
    global _NC
    if _NC is None:
        _NC = build_nc()
    return _NC


def kernel(enc_hiddens, dec_prev_hidden, W1_w, W1_b, W2_w, W2_b, V_w, V_b):
    from concourse import bass_utils

    nc = _get_nc()
    enc_hiddens = np.asarray(enc_hiddens, np.float32)
    dec_prev_hidden = np.asarray(dec_prev_hidden, np.float32)
    shared = {
        "W1_w": np.ascontiguousarray(W1_w, np.float32),
        "W1_b": np.ascontiguousarray(W1_b, np.float32),
        "W2_w": np.ascontiguousarray(W2_w, np.float32),
        "W2_b": np.ascontiguousarray(W2_b, np.float32),
        "V_w": np.ascontiguousarray(V_w, np.float32),
    }
    in_maps = []
    for i in range(NCORES):
        m = dict(shared)
        m["enc_hiddens"] = np.ascontiguousarray(enc_hiddens[i * BL:(i + 1) * BL])
        m["dec_prev_hidden"] = np.ascontiguousarray(
            dec_prev_hidden[i * BL:(i + 1) * BL])
        in_maps.append(m)

    res = bass_utils.run_bass_kernel_spmd(nc, in_maps,
                                          core_ids=list(range(NCORES)))
    outs = res.results
    context = np.concatenate([o["out_ctx"] for o in outs], axis=0).reshape(B, 1, E)
    attn = np.concatenate([o["out_attn"] for o in outs], axis=0)
    return context, attn


# revision 26
# speedup vs baseline: 1.2098x; 1.0892x over previous
"""Bahdanau-attention kernel for Trainium2, data-parallel over batch on 8 NeuronCores.

Per-core shard: 8 batches (1568 flat encoder rows). Design:
  - enc is read from HBM exactly once: SWDGE cast-DMA (f32->bf16) into
    resident SBUF "nat" tiles (one per chunk). PE-transposes produce encT
    (E on partitions) interleaved with the main matmuls so the PE stays warm.
  - W1/W2 are cast to bf16 DRAM staging slices, then loaded transposed via
    the xbar DMA-transpose (few, large instructions; no PE time).
  - Main loop per flat-row chunk [384,384,384,416]: bf16 matmuls accumulate
    enc_projT in f32 PSUM -> ScalarE tanh with per-partition bias
    Hb = h_projT + W1_b + W2_b -> V-dot matmul accumulates scores (f32).
  - Per completed batch: f32 softmax on its scores row; attn row DMA'd out;
    w row cast/scattered into the block-diagonal W8 [8 x 1568].
  - End: W8 column blocks are PE-transposed to wdiag [128 x 8] chunks; the
    context for all 8 batches accumulates in one [8, 512] PSUM per e-chunk
    from the resident nat tiles (enc never re-read).
"""

import sys

sys.path.insert(0, "/opt/trn_rl_repo")
sys.path.insert(0, "/opt/pypackages")

import numpy as np
from contextlib import ExitStack

import concourse.bass as bass
import concourse.bacc as bacc
import concourse.mybir as mybir
import concourse.tile as tile
from concourse.masks import make_identity

F32 = mybir.dt.float32
BF16 = mybir.dt.bfloat16
AF = mybir.ActivationFunctionType
ALU = mybir.AluOpType
AX = mybir.AxisListType

B, P, E, A = 64, 196, 2048, 1024
NCORES = 8
BL = B // NCORES          # 8 local batches per core
BP = BL * P               # 1568 flat rows per core
KE = E // 128             # 16 contraction chunks
KA = A // 128             # 8 attn-dim tiles
CHUNKS = [(0, 384), (384, 384), (768, 384), (1152, 416)]
CW = 416                  # max chunk width
NT = (BP + 127) // 128    # 13 flat 128-row tiles


def _segments(S, W):
    segs = []
    b = S // P
    pos = S
    while pos < S + W:
        end = min((b + 1) * P, S + W)
        segs.append((b, pos - S, end - pos))
        pos = end
        b += 1
    return segs


def _done_batches():
    done = []
    prev = 0
    for S, W in CHUNKS:
        nb = (S + W) // P
        done.append(list(range(prev, nb)))
        prev = nb
    return done


DONE = _done_batches()


def build_nc():
    nc = bacc.Bacc("TRN2", target_bir_lowering=False, debug=False,
                   enable_asserts=False)

    enc = nc.dram_tensor("enc_hiddens", [BL, P, E], F32, kind="ExternalInput")
    dec = nc.dram_tensor("dec_prev_hidden", [BL, E], F32, kind="ExternalInput")
    w1 = nc.dram_tensor("W1_w", [A, E], F32, kind="ExternalInput")
    w1b = nc.dram_tensor("W1_b", [A], F32, kind="ExternalInput")
    w2 = nc.dram_tensor("W2_w", [A, E], F32, kind="ExternalInput")
    w2b = nc.dram_tensor("W2_b", [A], F32, kind="ExternalInput")
    vw = nc.dram_tensor("V_w", [1, A], F32, kind="ExternalInput")
    out_ctx = nc.dram_tensor("out_ctx", [BL, E], F32, kind="ExternalOutput")
    out_attn = nc.dram_tensor("out_attn", [BL, P], F32, kind="ExternalOutput")

    enc_flat = enc.ap().rearrange("b p e -> (b p) e")

    with tile.TileContext(nc) as tc:
        with ExitStack() as ctx:
            const = ctx.enter_context(tc.tile_pool(name="const", bufs=1))
            identb = const.tile([128, 128], BF16)
            make_identity(nc, identb[:])

            # Persistent SBUF tensors
            w2t = const.tile([128, KE * A], BF16)       # W2^T: [e, k-major a]
            hb = const.tile([128, KA * BL], F32)        # Hb[q, a*BL+b]
            vt = const.tile([128, KA], BF16)            # V^T columns per a-tile
            scores = const.tile([1, BP], F32)
            w8 = const.tile([BL, BP], BF16)             # block-diag softmax w
            nc.vector.memset(w8[:], 0.0)
            # resident bf16 enc rows, one tile per chunk: [128, nj*E]
            nats = []
            for ci, (S, W) in enumerate(CHUNKS):
                nj = (W + 127) // 128
                nats.append(const.tile([128, nj * E], BF16, name=f"nat{ci}"))

            tp_ps = ctx.enter_context(
                tc.tile_pool(name="tp_ps", bufs=2, space="PSUM"))

            dramp = ctx.enter_context(
                tc.tile_pool(name="dram", bufs=1, space="DRAM"))
            w2stage = dramp.tile([A, E], BF16, tag="w2s", name="w2stage")
            w1stage = dramp.tile([A, E], BF16, tag="w1s", name="w1stage")

            # ---- cast-DMA issue order (one SWDGE queue, roughly serial) ----
            def cast_chunk(ci):
                S, W = CHUNKS[ci]
                nfull = W // 128
                nc.gpsimd.dma_start(
                    nats[ci][:].rearrange("p (j e) -> p j e", e=E)[:, :nfull, :],
                    enc_flat[S:S + nfull * 128, :]
                    .rearrange("(j p) e -> p j e", p=128))
                rem = W - nfull * 128
                if rem:
                    nc.gpsimd.dma_start(
                        nats[ci][:rem, nfull * E:nfull * E + E],
                        enc_flat[S + nfull * 128:S + W, :])

            dec_sb = const.tile([BL, E], BF16)
            nc.gpsimd.dma_start(dec_sb[:], dec.ap())
            with nc.allow_non_contiguous_dma(reason="tiny transposed load"):
                nc.gpsimd.dma_start(
                    vt[:], vw.ap().rearrange("o (a k) -> k (o a)", k=128))
            cast_chunk(0)
            nc.gpsimd.dma_start(w2stage[:], w2.ap())
            nc.gpsimd.dma_start(w1stage[:], w1.ap())
            cast_chunk(1)
            cast_chunk(2)
            cast_chunk(3)

            # enc transpose emission helper; chunks 0/1 are emitted before
            # h_proj so the PE has work while the weights stream in
            enctp = ctx.enter_context(tc.tile_pool(name="enct", bufs=2))
            encts = {}

            def do_transposes(ci):
                S, W = CHUNKS[ci]
                nat = nats[ci]
                enct = enctp.tile([128, KE * CW], tag="enct", name=f"enct{ci}",
                                  dtype=BF16)
                encts[ci] = enct
                nj = (W + 127) // 128
                for j in range(nj):
                    rows = min(128, W - j * 128)
                    for k in range(KE):
                        ps = tp_ps.tile([128, 128], BF16, tag="tp",
                                        name=f"tps{ci}_{j}_{k}")
                        nc.tensor.transpose(
                            ps[:, :rows],
                            nat[:rows, j * E + k * 128: j * E + (k + 1) * 128],
                            identb[:rows, :rows])
                        nc.vector.tensor_copy(
                            enct[:, k * CW + j * 128: k * CW + j * 128 + rows],
                            ps[:, :rows])

            do_transposes(0)
            do_transposes(1)

            # ---------------- setup ----------------
            with ExitStack() as sctx:
                sp1 = sctx.enter_context(tc.tile_pool(name="setup1", bufs=1))
                hps_pool = sctx.enter_context(
                    tc.tile_pool(name="hps", bufs=2, space="PSUM"))

                # W1T / W2T via DRAM->SBUF xbar DMA transpose (W1 first: Hb
                # is needed at the first tanh).
                w2t3 = w2t[:].rearrange("p (k a) -> p k a", k=KE)
                w1t = sp1.tile([128, KE * A], BF16)
                w1t3 = w1t[:].rearrange("p (k a) -> p k a", k=KE)
                for q in range(4):
                    nc.sync.dma_start_transpose(
                        w2t3[:, 4 * q: 4 * (q + 1), :],
                        w2stage[:, q * 512:(q + 1) * 512])
                for q in range(4):
                    nc.scalar.dma_start_transpose(
                        w1t3[:, 4 * q: 4 * (q + 1), :],
                        w1stage[:, q * 512:(q + 1) * 512])

                # dec -> decT (bf16)
                dect = sp1.tile([128, KE * BL], BF16)
                for k in range(KE):
                    ps = tp_ps.tile([128, 128], BF16, tag="tp")
                    nc.tensor.transpose(
                        ps[:, :BL], dec_sb[:, k * 128:(k + 1) * 128],
                        identb[:BL, :BL])
                    nc.any.tensor_copy(dect[:, k * BL:(k + 1) * BL], ps[:, :BL])

                # h_proj = dec @ W1.T  -> [BL, A] f32
                h_sb = sp1.tile([BL, A], F32)
                for half in range(2):
                    hps = hps_pool.tile([BL, 512], F32, tag="hps")
                    for k in range(KE):
                        nc.tensor.matmul(
                            hps[:],
                            dect[:, k * BL:(k + 1) * BL],
                            w1t[:, k * A + half * 512: k * A + half * 512 + 512],
                            start=(k == 0), stop=(k == KE - 1))
                    nc.any.tensor_copy(h_sb[:, half * 512:(half + 1) * 512],
                                       hps[:])

                # bias columns: W1_b + W2_b, laid out [q, a]  (f32)
                w1bc = sp1.tile([128, KA], F32)
                w2bc = sp1.tile([128, KA], F32)
                with nc.allow_non_contiguous_dma(reason="tiny transposed loads"):
                    nc.sync.dma_start(
                        w1bc[:], w1b.ap().rearrange("(a k) -> k a", k=128))
                    nc.sync.dma_start(
                        w2bc[:], w2b.ap().rearrange("(a k) -> k a", k=128))
                nc.vector.tensor_add(w1bc[:], w1bc[:], w2bc[:])

                # hT + bias -> Hb  (bf16 transpose of h_sb via PE)
                h_bf = sp1.tile([BL, A], BF16)
                nc.vector.tensor_copy(h_bf[:], h_sb[:])
                for a in range(KA):
                    ps = tp_ps.tile([128, 128], BF16, tag="tp")
                    nc.tensor.transpose(
                        ps[:, :BL], h_bf[:, a * 128:(a + 1) * 128],
                        identb[:BL, :BL])
                    nc.vector.tensor_scalar(
                        out=hb[:, a * BL:(a + 1) * BL], in0=ps[:, :BL],
                        scalar1=w1bc[:, a:a + 1], scalar2=None, op0=ALU.add)

            # ---------------- main loop ----------------
            tpool = ctx.enter_context(tc.tile_pool(name="tpool", bufs=3))
            smp = ctx.enter_context(tc.tile_pool(name="smp", bufs=2))
            bigps = ctx.enter_context(
                tc.tile_pool(name="bigps", bufs=3, space="PSUM"))
            scps = ctx.enter_context(
                tc.tile_pool(name="scps", bufs=1, space="PSUM"))

            for ci, (S, W) in enumerate(CHUNKS):
                if ci not in encts:
                    do_transposes(ci)
                enct = encts[ci]
                segs = _segments(S, W)
                sc = scps.tile([1, CW], F32, tag="sc")
                raws = []
                for a in range(KA):
                    ps = bigps.tile([128, CW], F32, tag="big")
                    for k in range(KE):
                        nc.tensor.matmul(
                            ps[:, :W],
                            w2t[:, k * A + a * 128: k * A + (a + 1) * 128],
                            enct[:, k * CW: k * CW + W],
                            start=(k == 0), stop=(k == KE - 1))
                    if ci == 0:
                        # Hb may not be ready yet; free the PSUM bank with a
                        # raw copy and apply tanh from SBUF once Hb lands.
                        raw = tpool.tile([128, CW], F32, tag=f"raw{a}",
                                         name=f"raw0_{a}")
                        nc.any.tensor_copy(raw[:, :W], ps[:, :W])
                        raws.append(raw)
                        continue
                    t_sb = tpool.tile([128, CW], BF16, tag="t")
                    for (b, off, ln) in segs:
                        nc.scalar.activation(
                            t_sb[:, off:off + ln], ps[:, off:off + ln], AF.Tanh,
                            bias=hb[:, a * BL + b: a * BL + b + 1])
                    nc.tensor.matmul(
                        sc[:, :W], vt[:, a:a + 1], t_sb[:, :W],
                        start=(a == 0), stop=(a == KA - 1))
                for a, raw in enumerate(raws):
                    t_sb = tpool.tile([128, CW], BF16, tag="t")
                    for (b, off, ln) in segs:
                        nc.scalar.activation(
                            t_sb[:, off:off + ln], raw[:, off:off + ln], AF.Tanh,
                            bias=hb[:, a * BL + b: a * BL + b + 1])
                    nc.tensor.matmul(
                        sc[:, :W], vt[:, a:a + 1], t_sb[:, :W],
                        start=(a == 0), stop=(a == KA - 1))
                nc.any.tensor_copy(scores[:, S:S + W], sc[:, :W])

                for b in DONE[ci]:
                    sseg = scores[:, b * P:(b + 1) * P]
                    negmax = smp.tile([1, 1], F32, tag="nm")
                    nc.vector.reduce_max(negmax[:], sseg, axis=AX.X, negate=True)
                    ex = smp.tile([1, P], F32, tag="ex")
                    nc.scalar.activation(ex[:], sseg, AF.Exp, bias=negmax[:])
                    ssum = smp.tile([1, 1], F32, tag="sm")
                    nc.vector.reduce_sum(ssum[:], ex[:], axis=AX.X)
                    rcp = smp.tile([1, 1], F32, tag="rc")
                    nc.vector.reciprocal(rcp[:], ssum[:])
                    wsb = smp.tile([1, P], F32, tag="w")
                    nc.vector.tensor_scalar(
                        out=wsb[:], in0=ex[:], scalar1=rcp[:], scalar2=None,
                        op0=ALU.mult)
                    nc.sync.dma_start(out_attn.ap()[b:b + 1, :], wsb[:])
                    # scatter w row (cast to bf16) onto the block diagonal
                    nc.gpsimd.dma_start(
                        w8[b:b + 1, b * P:(b + 1) * P], wsb[:])

            # ---------------- context (end phase) ----------------
            ctxps = ctx.enter_context(
                tc.tile_pool(name="ctxps", bufs=2, space="PSUM"))
            ctxp = ctx.enter_context(tc.tile_pool(name="ctxsb", bufs=1))
            wdiag = const.tile([128, NT * BL], BF16)
            for t in range(NT):
                rows = min(128, BP - t * 128)
                ps = tp_ps.tile([128, 128], BF16, tag="tp")
                nc.tensor.transpose(
                    ps[:rows, :BL], w8[:, t * 128: t * 128 + rows],
                    identb[:BL, :BL])
                nc.vector.tensor_copy(wdiag[:rows, t * BL:(t + 1) * BL],
                                      ps[:rows, :BL])

            # map flat tile t -> (chunk ci, j)
            tmap = []
            for ci, (S, W) in enumerate(CHUNKS):
                for j in range((W + 127) // 128):
                    tmap.append((ci, j))
            ctx_sb = ctxp.tile([BL, E], F32)
            for ec in range(4):
                cps = ctxps.tile([BL, 512], F32, tag="cps")
                for t in range(NT):
                    rows = min(128, BP - t * 128)
                    ci, j = tmap[t]
                    nc.tensor.matmul(
                        cps[:],
                        wdiag[:rows, t * BL:(t + 1) * BL],
                        nats[ci][:rows, j * E + ec * 512: j * E + (ec + 1) * 512],
                        start=(t == 0), stop=(t == NT - 1))
                nc.vector.tensor_copy(ctx_sb[:, ec * 512:(ec + 1) * 512], cps[:])
            nc.sync.dma_start(out_ctx.ap(), ctx_sb[:])

    nc.compile()
    return nc


_NC = None


def _get_nc():
    global _NC
    if _NC is None:
        _NC = build_nc()
    return _NC


def kernel(enc_hiddens, dec_prev_hidden, W1_w, W1_b, W2_w, W2_b, V_w, V_b):
    from concourse import bass_utils

    nc = _get_nc()
    enc_hiddens = np.asarray(enc_hiddens, np.float32)
    dec_prev_hidden = np.asarray(dec_prev_hidden, np.float32)
    shared = {
        "W1_w": np.ascontiguousarray(W1_w, np.float32),
        "W1_b": np.ascontiguousarray(W1_b, np.float32),
        "W2_w": np.ascontiguousarray(W2_w, np.float32),
        "W2_b": np.ascontiguousarray(W2_b, np.float32),
        "V_w": np.ascontiguousarray(V_w, np.float32),
    }
    in_maps = []
    for i in range(NCORES):
        m = dict(shared)
        m["enc_hiddens"] = np.ascontiguousarray(enc_hiddens[i * BL:(i + 1) * BL])
        m["dec_prev_hidden"] = np.ascontiguousarray(
            dec_prev_hidden[i * BL:(i + 1) * BL])
        in_maps.append(m)

    res = bass_utils.run_bass_kernel_spmd(nc, in_maps,
                                          core_ids=list(range(NCORES)))
    outs = res.results
    context = np.concatenate([o["out_ctx"] for o in outs], axis=0).reshape(B, 1, E)
    attn = np.concatenate([o["out_attn"] for o in outs], axis=0)
    return context, attn
